# revision 17
# baseline (speedup 1.0000x reference)
"""Trainium2 Bass kernel for nn_EnsembleTransitionModel — fp8 DoubleRow, 4-stage
software-pipelined edition.

Sharding: model-parallel. M=8 ensemble members across 8 NeuronCores; each core
runs one full MLP over the whole batch. Inputs replicated, weights sharded.

All matmuls are fp8e4 (e4m3) with MatmulPerfMode.DoubleRow (two 128-row
k-tiles per pass, 2x bf16 throughput). The PE instruction stream is software
pipelined four stages deep --

    iter i:  L1(i) | hidden0(i-1) | hidden1(i-2) | out(i-3)

-- so every stage's input activations were produced a full iteration (~13us)
earlier and the PE never waits on the Act/DVE engines (whose per-op latency is
~0.7us on [128,512] tiles). That also keeps the PE pstate clock pinned high.

Numerics: per-layer power-of-2 scale chain. Weights are host-scaled so their
fp8 values sit near sigma~1; each layer's psum carries gamma_l, activations
store alpha_l*h_l in fp8, and the ratio alpha_l/gamma_l is applied as an
immediate scale in the activation op. BatchNorm (eval) scale folds into the
next weight matrix's columns on host; BN bias rides the Act-engine bias
operand; L1's b1 rides a constant-1.0 row in x's padding (row 1925) so the
L1 relu needs no bias and runs on the DVE as one tensor_scalar (mult, max).
The output stage writes 8192*(delta+z+b3) in bf16; the host divides once.
"""

import os
import sys

import numpy as np

for _p in ("/opt/trn_rl_repo", "/root/.axon_site/_ro/trn_rl_repo"):
    if os.path.isdir(_p) and _p not in sys.path:
        sys.path.insert(0, _p)

M = 8
B = 16384
HIST = 5
L = 384
A = 1
HID = 512
NHL = 2
DIN = L * HIST + A * HIST  # 1925
EPS = 1e-5

NCH = 512  # batch columns per chunk (= 1 PSUM bank of fp32)
KT = 16  # x^T padded to 2048 rows (1925 real + ones-row 1925 for b1)
NP1 = KT // 2  # 8 DoubleRow k-tile pairs for layer 1
HT = HID // 128  # 4 hidden feature tiles
HPAIR = HT // 2  # 2 DoubleRow pairs
LT = L // 128  # 3 output feature tiles
ZROW0 = (HIST - 1) * L  # 1536: first row of z_last within x^T
ONEROW = DIN  # 1925: constant-1.0 row carrying b1 into the L1 psum

# per-layer scale chain: psum_l = G_l * (W_l^T h_{l-1}); stored h_l = A_l * h_l
G1, A1 = 64.0, 16.0
G2, A2 = 1024.0, 64.0
G3, A3 = 4096.0, 128.0
G4 = 8192.0  # output psum & residual scale; host divides by G4
SC_L1 = A1 / G1  # immediate scale in the L1 DVE relu
SC_H = (A2 / G2, A3 / G3)  # immediate scales in the hidden Act relus

# vecs columns: [A2*c0 (4) | A3*c1 (4)]
COL_C = lambda l: 4 * l
NVEC = 4 * NHL


def build_bass(batch=B):
    import concourse.bacc as bacc
    import concourse.tile as tile
    from concourse import mybir

    f32 = mybir.dt.float32
    bf16 = mybir.dt.bfloat16
    fp8 = mybir.dt.float8e4
    DR = mybir.MatmulPerfMode.DoubleRow
    Relu = mybir.ActivationFunctionType.Relu
    add = mybir.AluOpType.add
    mult = mybir.AluOpType.mult
    amax = mybir.AluOpType.max

    nch = batch // NCH
    assert nch * NCH == batch

    nc = bacc.Bacc("TRN2", target_bir_lowering=False)
    x8 = nc.declare_dram_parameter("x8", [nch, 128, KT, NCH], fp8, isOutput=False)
    zg = nc.declare_dram_parameter("zg", [nch, 128, LT, NCH], bf16, isOutput=False)
    w1 = nc.declare_dram_parameter("w1", [128, KT, HID], fp8, isOutput=False)
    wh = nc.declare_dram_parameter(
        "wh", [128, NHL, HPAIR, 2, HT, 128], fp8, isOutput=False
    )
    w3 = nc.declare_dram_parameter("w3", [128, HPAIR, 2, L], fp8, isOutput=False)
    vecs = nc.declare_dram_parameter("vecs", [128, NVEC], f32, isOutput=False)
    # chunk-blocked so each chunk's output is one fully-contiguous 384KB DMA
    outc = nc.declare_dram_parameter("outc", [nch, 128, LT, NCH], bf16, isOutput=True)

    with tile.TileContext(nc) as tc:
        with (
            tc.tile_pool(name="wt", bufs=1) as wpool,
            tc.tile_pool(name="x", bufs=3) as xpool,
            tc.tile_pool(name="z", bufs=6) as zpool,
            tc.tile_pool(name="h", bufs=3) as hpool,
            tc.tile_pool(name="o", bufs=3) as opool,
            tc.tile_pool(name="ps1", bufs=4, space="PSUM") as ps1pool,
            tc.tile_pool(name="psh", bufs=4, space="PSUM") as pshpool,
        ):
            w1_sb = wpool.tile([128, KT, HID], fp8, tag="w1")
            wh_sb = wpool.tile([128, NHL, HPAIR, 2, HT, 128], fp8, tag="wh")
            w3_sb = wpool.tile([128, HPAIR, 2, L], fp8, tag="w3")
            v_sb = wpool.tile([128, NVEC], f32, tag="vecs")

            xts, zts, h1s, h2s, h3s = {}, {}, {}, {}, {}

            def dma_x(c, per_pair=False):
                xt = xpool.tile([128, KT, NCH], fp8, tag="x", name="xt")
                if per_pair:
                    # chunk 0: pair 0 rides its own small DMAs so the first
                    # matmul gates on ~256KB; the rest follows in two bulk
                    # transfers (fewer serial dma_start issues on Sync)
                    nc.sync.dma_start(out=w1_sb[:, 0:2, :], in_=w1[:, 0:2, :])
                    nc.sync.dma_start(out=xt[:, 0:2, :], in_=x8[c, :, 0:2, :])
                    nc.sync.dma_start(out=w1_sb[:, 2:KT, :], in_=w1[:, 2:KT, :])
                    nc.sync.dma_start(out=xt[:, 2:KT, :], in_=x8[c, :, 2:KT, :])
                else:
                    nc.sync.dma_start(out=xt[:], in_=x8[c, :, :, :])
                xts[c] = xt

            def dma_z(c):
                zt = zpool.tile([128, LT, NCH], bf16, tag="z", name="zt")
                nc.sync.dma_start(out=zt[:], in_=zg[c, :, :, :])
                zts[c] = zt

            def stage_l1(c):
                xt = xts.pop(c)
                h1p = [
                    hpool.tile([128, 2, NCH], fp8, tag=f"h1p{j}", name=f"h1p{j}")
                    for j in range(HPAIR)
                ]
                for ht in range(HT):
                    ps = ps1pool.tile([128, NCH], f32, tag="ps1", name="ps")
                    for pr in range(NP1):
                        nc.tensor.matmul(
                            ps[:],
                            w1_sb[:, 2 * pr : 2 * pr + 2, ht * 128 : (ht + 1) * 128],
                            xt[:, 2 * pr : 2 * pr + 2, :],
                            start=(pr == 0),
                            stop=(pr == NP1 - 1),
                            perf_mode=DR,
                        )
                    # relu on the DVE: h1 = max(SC_L1 * psum, 0); b1 already in
                    # psum via the ones-row
                    nc.vector.tensor_scalar(
                        h1p[ht // 2][:, ht % 2, :], ps[:], SC_L1, 0.0, mult, amax
                    )
                h1s[c] = h1p

            def stage_hidden(l, src, dst, c):
                hin = src.pop(c)
                hout = [
                    hpool.tile(
                        [128, 2, NCH], fp8, tag=f"h{l + 2}p{j}", name=f"h{l + 2}p{j}"
                    )
                    for j in range(HPAIR)
                ]
                for mt in range(HT):
                    ps = pshpool.tile([128, NCH], f32, tag="ps2", name="ps")
                    for pr in range(HPAIR):
                        nc.tensor.matmul(
                            ps[:],
                            wh_sb[:, l, pr, :, mt, :],
                            hin[pr][:],
                            start=(pr == 0),
                            stop=(pr == HPAIR - 1),
                            perf_mode=DR,
                        )
                    nc.scalar.activation(
                        hout[mt // 2][:, mt % 2, :],
                        ps[:],
                        Relu,
                        bias=v_sb[:, COL_C(l) + mt : COL_C(l) + mt + 1],
                        scale=SC_H[l],
                    )
                dst[c] = hout

            def stage_out(c):
                hin = h3s.pop(c)
                zt = zts.pop(c)
                ot = opool.tile([128, LT, NCH], bf16, tag="ot", name="ot")
                for lt in range(LT):
                    ps = ps1pool.tile([128, NCH], f32, tag="ps1", name="ps")
                    for pr in range(HPAIR):
                        nc.tensor.matmul(
                            ps[:],
                            w3_sb[:, pr, :, lt * 128 : (lt + 1) * 128],
                            hin[pr][:],
                            start=(pr == 0),
                            stop=(pr == HPAIR - 1),
                            perf_mode=DR,
                        )
                    nc.vector.tensor_tensor(ot[:, lt, :], ps[:], zt[:, lt, :], add)
                # partition-split: SBUF rows stay contiguous (full 3KB), DRAM
                # stays contiguous per 96KB slice -> 4 DMA engines in parallel
                for p0 in range(0, 128, 32):
                    nc.sync.dma_start(
                        out=outc[c, p0 : p0 + 32, :, :], in_=ot[p0 : p0 + 32, :, :]
                    )

            dma_x(0, per_pair=True)
            for i in range(nch + 3):
                if i + 1 < nch:
                    dma_x(i + 1)
                if i == 0:
                    nc.sync.dma_start(out=v_sb[:], in_=vecs[:])
                    nc.sync.dma_start(out=wh_sb[:], in_=wh[:])
                    nc.sync.dma_start(out=w3_sb[:], in_=w3[:])
                if i < nch:
                    dma_z(i)
                if i < nch:
                    stage_l1(i)
                if 1 <= i < nch + 1:
                    stage_hidden(0, h1s, h2s, i - 1)
                if 2 <= i < nch + 2:
                    stage_hidden(1, h2s, h3s, i - 2)
                if i >= 3:
                    stage_out(i - 3)
    nc.compile()
    return nc


def prep_core_inputs(
    z_hist, a_hist, W1, b1, Wh, bh, gamma, beta, rmean, rvar, W3, b3
):
    """Host-side shard prep: returns per-model input dicts (x8 shared)."""
    import ml_dtypes

    fp8 = ml_dtypes.float8_e4m3
    bf16 = ml_dtypes.bfloat16
    batch = z_hist.shape[0]
    nch = batch // NCH
    x = np.concatenate(
        [z_hist.reshape(batch, -1), a_hist.reshape(batch, -1)], axis=1
    ).astype(np.float32)
    xpadT = np.zeros((KT * 128, batch), np.float32)
    xpadT[:DIN] = x.T
    xpadT[ONEROW] = 1.0  # carries b1 through the L1 matmul
    x8 = np.ascontiguousarray(
        xpadT.reshape(KT, 128, nch, NCH).transpose(2, 1, 0, 3)
    ).astype(fp8)
    z_lastT = xpadT[ZROW0 : ZROW0 + L]  # [L, batch] f32

    rstd = 1.0 / np.sqrt(rvar.astype(np.float64) + EPS)  # [NHL, M, HID]
    s_aff = (gamma * rstd).astype(np.float32)
    c_aff = ((bh - rmean) * gamma * rstd + beta).astype(np.float32)

    in_maps = []
    for m in range(M):
        w1p = np.zeros((KT * 128, HID), np.float32)
        w1p[:DIN] = W1[m] * G1
        w1p[ONEROW] = b1[m] * G1
        w1h = np.ascontiguousarray(
            w1p.reshape(KT, 128, HID).transpose(1, 0, 2)
        ).astype(fp8)  # [128, KT, HID]

        # BN scale of layer l folds into Wh[l]'s columns; weight rows absorb
        # the previous layer's stored-activation scale A_{l-1}
        whs = np.stack(
            [
                Wh[0, m] * s_aff[0, m][None, :] * (G2 / A1),
                Wh[1, m] * s_aff[1, m][None, :] * (G3 / A2),
            ]
        )
        whh = np.ascontiguousarray(
            whs.reshape(NHL, HPAIR, 2, 128, HT, 128).transpose(3, 0, 1, 2, 4, 5)
        ).astype(fp8)  # [128, NHL, pr, i, mt, 128]

        w3h = np.ascontiguousarray(
            (W3[m] * (G4 / A3)).reshape(HPAIR, 2, 128, L).transpose(2, 0, 1, 3)
        ).astype(fp8)  # [128, pr, i, L]

        vecs = np.zeros((128, NVEC), np.float32)
        vecs[:, COL_C(0) : COL_C(0) + HT] = (c_aff[0, m] * A2).reshape(HT, 128).T
        vecs[:, COL_C(1) : COL_C(1) + HT] = (c_aff[1, m] * A3).reshape(HT, 128).T

        zgm = np.ascontiguousarray(
            ((z_lastT + b3[m][:, None]) * G4)
            .reshape(LT, 128, nch, NCH)
            .transpose(2, 1, 0, 3)
        ).astype(bf16)  # [nch, 128, LT, NCH]

        in_maps.append(
            {"x8": x8, "zg": zgm, "w1": w1h, "wh": whh, "w3": w3h, "vecs": vecs}
        )
    return in_maps


def postprocess(results):
    """[M dicts with outc [nch, 128, LT, NCH] bf16 * G4] -> [M, batch, L] f32."""
    outs = []
    for m in range(M):
        a = results[m]["outc"].astype(np.float32) / G4  # [nch, 128, LT, NCH]
        outs.append(a.transpose(0, 3, 2, 1).reshape(-1, L))  # [batch, L]
    return np.stack(outs)


def _reset_device():
    """Clear any exec-unit wedge a previous (profiled) session left behind."""
    try:
        import ctypes

        import jax

        jax.devices()
        lib = ctypes.CDLL("/opt/axon/libaxon_pjrt.so")
        if hasattr(lib, "axon_reset"):
            lib.axon_reset.restype = ctypes.c_int64
            lib.axon_reset()
    except Exception:
        pass


def kernel(**inputs):
    inputs = {k: np.asarray(v) for k, v in inputs.items()}
    in_maps = prep_core_inputs(**inputs)
    nc = build_bass(B)

    from concourse import bass_utils

    _reset_device()
    res = bass_utils.run_bass_kernel_spmd(nc, in_maps, core_ids=list(range(M)))
    return postprocess(res.results)


# revision 19
# speedup vs baseline: 1.0317x; 1.0317x over previous
"""Trainium2 Bass kernel for nn_EnsembleTransitionModel — fp8 DoubleRow, 4-stage
software-pipelined edition.

Sharding: model-parallel. M=8 ensemble members across 8 NeuronCores; each core
runs one full MLP over the whole batch. Inputs replicated, weights sharded.

All matmuls are fp8e4 (e4m3) with MatmulPerfMode.DoubleRow (two 128-row
k-tiles per pass, 2x bf16 throughput). The PE instruction stream is software
pipelined four stages deep --

    iter i:  L1(i) | hidden0(i-1) | hidden1(i-2) | out(i-3)

-- so every stage's input activations were produced a full iteration (~13us)
earlier and the PE never waits on the Act/DVE engines (whose per-op latency is
~0.7us on [128,512] tiles). That also keeps the PE pstate clock pinned high.

Numerics: per-layer power-of-2 scale chain. Weights are host-scaled so their
fp8 values sit near sigma~1; each layer's psum carries gamma_l, activations
store alpha_l*h_l in fp8, and the ratio alpha_l/gamma_l is applied as an
immediate scale in the activation op. BatchNorm (eval) scale folds into the
next weight matrix's columns on host; BN bias rides the Act-engine bias
operand; L1's b1 rides a constant-1.0 row in x's padding (row 1925) so the
L1 relu needs no bias and runs on the DVE as one tensor_scalar (mult, max).
The output stage writes 8192*(delta+z+b3) in bf16; the host divides once.
"""

import os
import sys

import numpy as np

for _p in ("/opt/trn_rl_repo", "/root/.axon_site/_ro/trn_rl_repo"):
    if os.path.isdir(_p) and _p not in sys.path:
        sys.path.insert(0, _p)

M = 8
B = 16384
HIST = 5
L = 384
A = 1
HID = 512
NHL = 2
DIN = L * HIST + A * HIST  # 1925
EPS = 1e-5

NCH = 512  # batch columns per chunk (= 1 PSUM bank of fp32)
KT = 16  # x^T padded to 2048 rows (1925 real + ones-row 1925 for b1)
NP1 = KT // 2  # 8 DoubleRow k-tile pairs for layer 1
HT = HID // 128  # 4 hidden feature tiles
HPAIR = HT // 2  # 2 DoubleRow pairs
LT = L // 128  # 3 output feature tiles
ZROW0 = (HIST - 1) * L  # 1536: first row of z_last within x^T
ONEROW = DIN  # 1925: constant-1.0 row carrying b1 into the L1 psum

# per-layer scale chain: psum_l = G_l * (W_l^T h_{l-1}); stored h_l = A_l * h_l
G1, A1 = 64.0, 16.0
G2, A2 = 1024.0, 64.0
G3, A3 = 4096.0, 128.0
G4 = 8192.0  # output psum & residual scale; host divides by G4
SC_L1 = A1 / G1  # immediate scale in the L1 DVE relu
SC_H = (A2 / G2, A3 / G3)  # immediate scales in the hidden Act relus

# vecs columns: [A2*c0 (4) | A3*c1 (4)]
COL_C = lambda l: 4 * l
NVEC = 4 * NHL


def build_bass(batch=B):
    import concourse.bacc as bacc
    import concourse.tile as tile
    from concourse import mybir

    f32 = mybir.dt.float32
    bf16 = mybir.dt.bfloat16
    fp8 = mybir.dt.float8e4
    DR = mybir.MatmulPerfMode.DoubleRow
    Relu = mybir.ActivationFunctionType.Relu
    add = mybir.AluOpType.add
    mult = mybir.AluOpType.mult
    amax = mybir.AluOpType.max

    nch = batch // NCH
    assert nch * NCH == batch

    nc = bacc.Bacc("TRN2", target_bir_lowering=False)
    x8 = nc.declare_dram_parameter("x8", [nch, 128, KT, NCH], fp8, isOutput=False)
    zg = nc.declare_dram_parameter("zg", [nch, 128, LT, NCH], bf16, isOutput=False)
    w1 = nc.declare_dram_parameter("w1", [128, KT, HID], fp8, isOutput=False)
    wh = nc.declare_dram_parameter(
        "wh", [128, NHL, HPAIR, 2, HT, 128], fp8, isOutput=False
    )
    w3 = nc.declare_dram_parameter("w3", [128, HPAIR, 2, L], fp8, isOutput=False)
    vecs = nc.declare_dram_parameter("vecs", [128, NVEC], f32, isOutput=False)
    # chunk-blocked so each chunk's output is one fully-contiguous 384KB DMA
    outc = nc.declare_dram_parameter("outc", [nch, 128, LT, NCH], bf16, isOutput=True)

    with tile.TileContext(nc) as tc:
        with (
            tc.tile_pool(name="wt", bufs=1) as wpool,
            tc.tile_pool(name="x", bufs=3) as xpool,
            tc.tile_pool(name="z", bufs=6) as zpool,
            tc.tile_pool(name="h", bufs=3) as hpool,
            tc.tile_pool(name="o", bufs=3) as opool,
            tc.tile_pool(name="ps1", bufs=4, space="PSUM") as ps1pool,
            tc.tile_pool(name="psh", bufs=4, space="PSUM") as pshpool,
        ):
            w1_sb = wpool.tile([128, KT, HID], fp8, tag="w1")
            wh_sb = wpool.tile([128, NHL, HPAIR, 2, HT, 128], fp8, tag="wh")
            w3_sb = wpool.tile([128, HPAIR, 2, L], fp8, tag="w3")
            v_sb = wpool.tile([128, NVEC], f32, tag="vecs")

            xts, zts, h1s, h2s, h3s = {}, {}, {}, {}, {}

            def dma_x(c, per_pair=False):
                xt = xpool.tile([128, KT, NCH], fp8, tag="x", name="xt")
                if per_pair:
                    # chunk 0: interleave w1/x pair slices so the first matmul
                    # group starts after ~256KB instead of the full preload
                    for pr in range(NP1):
                        nc.sync.dma_start(
                            out=w1_sb[:, 2 * pr : 2 * pr + 2, :],
                            in_=w1[:, 2 * pr : 2 * pr + 2, :],
                        )
                        nc.sync.dma_start(
                            out=xt[:, 2 * pr : 2 * pr + 2, :],
                            in_=x8[c, :, 2 * pr : 2 * pr + 2, :],
                        )
                else:
                    nc.sync.dma_start(out=xt[:], in_=x8[c, :, :, :])
                xts[c] = xt

            def dma_z(c):
                zt = zpool.tile([128, LT, NCH], bf16, tag="z", name="zt")
                nc.sync.dma_start(out=zt[:], in_=zg[c, :, :, :])
                zts[c] = zt

            def stage_l1(c):
                xt = xts.pop(c)
                h1p = [
                    hpool.tile([128, 2, NCH], fp8, tag=f"h1p{j}", name=f"h1p{j}")
                    for j in range(HPAIR)
                ]
                for ht in range(HT):
                    ps = ps1pool.tile([128, NCH], f32, tag="ps1", name="ps")
                    for pr in range(NP1):
                        nc.tensor.matmul(
                            ps[:],
                            w1_sb[:, 2 * pr : 2 * pr + 2, ht * 128 : (ht + 1) * 128],
                            xt[:, 2 * pr : 2 * pr + 2, :],
                            start=(pr == 0),
                            stop=(pr == NP1 - 1),
                            perf_mode=DR,
                        )
                    # relu on the DVE: h1 = max(SC_L1 * psum, 0); b1 already in
                    # psum via the ones-row
                    nc.vector.tensor_scalar(
                        h1p[ht // 2][:, ht % 2, :], ps[:], SC_L1, 0.0, mult, amax
                    )
                h1s[c] = h1p

            def stage_hidden(l, src, dst, c):
                hin = src.pop(c)
                hout = [
                    hpool.tile(
                        [128, 2, NCH], fp8, tag=f"h{l + 2}p{j}", name=f"h{l + 2}p{j}"
                    )
                    for j in range(HPAIR)
                ]
                for mt in range(HT):
                    ps = pshpool.tile([128, NCH], f32, tag="ps2", name="ps")
                    for pr in range(HPAIR):
                        nc.tensor.matmul(
                            ps[:],
                            wh_sb[:, l, pr, :, mt, :],
                            hin[pr][:],
                            start=(pr == 0),
                            stop=(pr == HPAIR - 1),
                            perf_mode=DR,
                        )
                    nc.scalar.activation(
                        hout[mt // 2][:, mt % 2, :],
                        ps[:],
                        Relu,
                        bias=v_sb[:, COL_C(l) + mt : COL_C(l) + mt + 1],
                        scale=SC_H[l],
                    )
                dst[c] = hout

            def stage_out(c):
                hin = h3s.pop(c)
                zt = zts.pop(c)
                ot = opool.tile([128, LT, NCH], bf16, tag="ot", name="ot")
                for lt in range(LT):
                    ps = ps1pool.tile([128, NCH], f32, tag="ps1", name="ps")
                    for pr in range(HPAIR):
                        nc.tensor.matmul(
                            ps[:],
                            w3_sb[:, pr, :, lt * 128 : (lt + 1) * 128],
                            hin[pr][:],
                            start=(pr == 0),
                            stop=(pr == HPAIR - 1),
                            perf_mode=DR,
                        )
                    nc.vector.tensor_tensor(ot[:, lt, :], ps[:], zt[:, lt, :], add)
                nc.sync.dma_start(out=outc[c, :, :, :], in_=ot[:])

            dma_x(0, per_pair=True)
            for i in range(nch + 3):
                if i + 1 < nch:
                    dma_x(i + 1)
                if i == 0:
                    nc.sync.dma_start(out=v_sb[:], in_=vecs[:])
                    nc.sync.dma_start(out=wh_sb[:], in_=wh[:])
                    nc.sync.dma_start(out=w3_sb[:], in_=w3[:])
                if i < nch:
                    dma_z(i)
                if i < nch:
                    stage_l1(i)
                if 1 <= i < nch + 1:
                    stage_hidden(0, h1s, h2s, i - 1)
                if 2 <= i < nch + 2:
                    stage_hidden(1, h2s, h3s, i - 2)
                if i >= 3:
                    stage_out(i - 3)
    nc.compile()
    return nc


def prep_core_inputs(
    z_hist, a_hist, W1, b1, Wh, bh, gamma, beta, rmean, rvar, W3, b3
):
    """Host-side shard prep: returns per-model input dicts (x8 shared)."""
    import ml_dtypes

    fp8 = ml_dtypes.float8_e4m3
    bf16 = ml_dtypes.bfloat16
    batch = z_hist.shape[0]
    nch = batch // NCH
    x = np.concatenate(
        [z_hist.reshape(batch, -1), a_hist.reshape(batch, -1)], axis=1
    ).astype(np.float32)
    xpadT = np.zeros((KT * 128, batch), np.float32)
    xpadT[:DIN] = x.T
    xpadT[ONEROW] = 1.0  # carries b1 through the L1 matmul
    x8 = np.ascontiguousarray(
        xpadT.reshape(KT, 128, nch, NCH).transpose(2, 1, 0, 3)
    ).astype(fp8)
    z_lastT = xpadT[ZROW0 : ZROW0 + L]  # [L, batch] f32

    rstd = 1.0 / np.sqrt(rvar.astype(np.float64) + EPS)  # [NHL, M, HID]
    s_aff = (gamma * rstd).astype(np.float32)
    c_aff = ((bh - rmean) * gamma * rstd + beta).astype(np.float32)

    in_maps = []
    for m in range(M):
        w1p = np.zeros((KT * 128, HID), np.float32)
        w1p[:DIN] = W1[m] * G1
        w1p[ONEROW] = b1[m] * G1
        w1h = np.ascontiguousarray(
            w1p.reshape(KT, 128, HID).transpose(1, 0, 2)
        ).astype(fp8)  # [128, KT, HID]

        # BN scale of layer l folds into Wh[l]'s columns; weight rows absorb
        # the previous layer's stored-activation scale A_{l-1}
        whs = np.stack(
            [
                Wh[0, m] * s_aff[0, m][None, :] * (G2 / A1),
                Wh[1, m] * s_aff[1, m][None, :] * (G3 / A2),
            ]
        )
        whh = np.ascontiguousarray(
            whs.reshape(NHL, HPAIR, 2, 128, HT, 128).transpose(3, 0, 1, 2, 4, 5)
        ).astype(fp8)  # [128, NHL, pr, i, mt, 128]

        w3h = np.ascontiguousarray(
            (W3[m] * (G4 / A3)).reshape(HPAIR, 2, 128, L).transpose(2, 0, 1, 3)
        ).astype(fp8)  # [128, pr, i, L]

        vecs = np.zeros((128, NVEC), np.float32)
        vecs[:, COL_C(0) : COL_C(0) + HT] = (c_aff[0, m] * A2).reshape(HT, 128).T
        vecs[:, COL_C(1) : COL_C(1) + HT] = (c_aff[1, m] * A3).reshape(HT, 128).T

        zgm = np.ascontiguousarray(
            ((z_lastT + b3[m][:, None]) * G4)
            .reshape(LT, 128, nch, NCH)
            .transpose(2, 1, 0, 3)
        ).astype(bf16)  # [nch, 128, LT, NCH]

        in_maps.append(
            {"x8": x8, "zg": zgm, "w1": w1h, "wh": whh, "w3": w3h, "vecs": vecs}
        )
    return in_maps


def postprocess(results):
    """[M dicts with outc [nch, 128, LT, NCH] bf16 * G4] -> [M, batch, L] f32."""
    outs = []
    for m in range(M):
        a = results[m]["outc"].astype(np.float32) / G4  # [nch, 128, LT, NCH]
        outs.append(a.transpose(0, 3, 2, 1).reshape(-1, L))  # [batch, L]
    return np.stack(outs)


def _reset_device():
    """Clear any exec-unit wedge a previous (profiled) session left behind."""
    try:
        import ctypes

        import jax

        jax.devices()
        lib = ctypes.CDLL("/opt/axon/libaxon_pjrt.so")
        if hasattr(lib, "axon_reset"):
            lib.axon_reset.restype = ctypes.c_int64
            lib.axon_reset()
    except Exception:
        pass


def kernel(**inputs):
    inputs = {k: np.asarray(v) for k, v in inputs.items()}
    in_maps = prep_core_inputs(**inputs)
    nc = build_bass(B)

    from concourse import bass_utils

    _reset_device()
    res = bass_utils.run_bass_kernel_spmd(nc, in_maps, core_ids=list(range(M)))
    return postprocess(res.results)
